# revision 27
# baseline (speedup 1.0000x reference)
"""Trainium2 Bass kernel for ragged masked attention-score softmax.

Problem (B=32, T=8192, H=128):
    energy[b,t] = relu(W1 @ hidden[b] + W2 @ enc[t,b] + b_attn)   (W_attn = [W1 | W2])
    scores[b,t] = v . energy[b,t]
    out[b,0,:]  = ragged-masked softmax over t < len_seq[b], zeros after.

Strategy (8 NeuronCores, flat position-parallel):
  - Every row is padded to a 512-position multiple and the resulting stream of
    512-position "groups" is dealt evenly across the 8 cores (NG groups each,
    rows may straddle cores).  All cores run one shared graph.
  - Host precomputes hproj[r] = W1 @ hidden[r] + b_attn (f64) and ships one
    f32 bias column per group, so the device never touches W1/hidden.
  - Device per group: energy = w2t.T @ encT (PE, bf16) -> bias+relu on
    ScalarE/VectorE (alternating per group) -> per-128-block v-dot via PE
    (energy block stationary, v moving) accumulating scores into one PSUM
    tile [128, 4*NG], position-major.
  - Scores are bounded (|s| <~ 8) so no max subtraction is needed: the device
    applies exp directly (f32) and DMAs the raw exp values out.  The host
    gather divides each row by its f64 sum - exact softmax, and rows split
    across cores combine for free.
  - A ~3.4us chain of dummy matmuls at graph start releases the PE HAM clock
    gate (1.2 -> 2.4 GHz) during the initial DMA window, so the real stream
    runs warm from its first instruction.
"""

from contextlib import ExitStack

import numpy as np

import concourse.bass as bass
import concourse.tile as tile
from concourse import bacc, mybir
from concourse.bass_utils import run_bass_kernel_spmd

B, T, H = 32, 8192, 128
NCORES = 8
GRP = 512  # positions per group (max moving free dim / PSUM bank)
VLAG = 2  # groups of lag between relu and v-dot emission
NWARM = 9  # dummy warm-up matmuls (~9 * 427ns cold = 3.8us)
NOUT = 4  # exp/output DMA pieces
CHW = 1024  # enc DMA chunk width (columns)


def _np_bf16():
    import ml_dtypes

    return np.dtype(ml_dtypes.bfloat16)


def _plan(ls):
    """Lay out the global stream of 512-position groups and deal to cores.

    Returns (NG, core_groups) where core_groups[k] is a list of length NG of
    (row, j) tuples - group j of row `row` covers positions [512j, 512j+512)
    - with None entries for padding groups.
    """
    glist = []
    for r in range(B):
        for j in range((int(ls[r]) + GRP - 1) // GRP):
            glist.append((r, j))
    ng_tot = len(glist)
    NG = (ng_tot + NCORES - 1) // NCORES
    npad = NG * NCORES - ng_tot
    # spread the padding groups over the tails of the last `npad` cores
    core_groups = []
    pos = 0
    for k in range(NCORES):
        take = NG - (1 if k >= NCORES - npad else 0)
        gs = glist[pos : pos + take] + [None] * (NG - take)
        pos += take
        core_groups.append(gs)
    return NG, core_groups


def _build(nc, NG):
    """Emit the shared Tile graph for one core (NG groups of 512 positions)."""
    bf16 = mybir.dt.bfloat16
    f32 = mybir.dt.float32
    AF = mybir.ActivationFunctionType
    NB = 4 * NG  # number of 128-position blocks
    CC = H + 1  # const columns (w2t | v) packed in front of enc
    NC = CC + GRP * NG  # total enc columns incl consts

    enc = nc.dram_tensor("enc", [H, NC], bf16, kind="ExternalInput").ap()
    hpb = nc.dram_tensor("hpb", [128, NG], f32, kind="ExternalInput").ap()
    out = nc.dram_tensor("out", [128, NB], f32, kind="ExternalOutput").ap()

    with ExitStack() as ctx:
        tc = ctx.enter_context(tile.TileContext(nc))
        singles = ctx.enter_context(tc.tile_pool(name="singles", bufs=1))
        enpool = ctx.enter_context(tc.tile_pool(name="energy", bufs=8))
        ps_e = ctx.enter_context(tc.tile_pool(name="ps_e", bufs=5, space="PSUM"))
        ps_sc = ctx.enter_context(tc.tile_pool(name="ps_sc", bufs=1, space="PSUM"))
        ps_w = ctx.enter_context(tc.tile_pool(name="ps_w", bufs=1, space="PSUM"))

        # ---- PE warm-up: ~3.4us of dense dummy matmuls during the DMA wait
        # window flips the HAM clock gate to 2.4 GHz before the real stream.
        dum = singles.tile([H, H], bf16)
        nc.vector.memset(dum[:], 0.0)
        dumr = singles.tile([H, GRP], bf16)
        nc.vector.memset(dumr[:], 0.0)
        pw = ps_w.tile([H, GRP], f32, tag="warm")
        for _ in range(NWARM):
            nc.tensor.matmul(out=pw[:], lhsT=dum[:], rhs=dumr[:], start=True, stop=True)

        # ---- DMAs: uniform chunks at a steady cadence; the consts ride in
        # front of chunk 0 so one semaphore covers both.  Compute is paced to
        # trail the chunk-semaphore wave by a constant margin (see NWARM).
        enc_sb = singles.tile([H, NC], bf16)
        w2t_sb = enc_sb[:, :H]
        vvec_sb = enc_sb[:, H : H + 1]
        # small chunks early (dense, early semaphore wave for the pipeline
        # head), wider chunks later where the wave is latency-pipelined, and
        # short tail chunks for a quick final semaphore
        chunks = []
        c0 = 0
        while c0 < NC:
            rem = NC - c0
            if c0 == 0:
                cw = CC + GRP
            elif len(chunks) < 8:
                cw = min(GRP, rem)
            elif rem <= 2 * GRP:
                cw = min(GRP, rem)
            else:
                cw = min(CHW, rem - 2 * GRP)
            chunks.append((c0, cw))
            c0 += cw
        # the first few issues alternate Scalar/Sync HWDGE rings: each ring's
        # ~0.6us per-issue cost would otherwise cap the early semaphore wave
        hpb_sb = singles.tile([128, NG], f32)
        for i, (c0, cw) in enumerate(chunks):
            eng = nc.scalar if (i < 8 and i % 2 == 0) else nc.sync
            eng.dma_start(enc_sb[:, c0 : c0 + cw], enc[:, c0 : c0 + cw])
            if i == 0:
                nc.scalar.dma_start(hpb_sb[:], hpb[:])
                # preload the exp ACT table (1.3us) after ScalarE's first DMA
                # issues but before its first relu, overlapping the enc stream
                dume = singles.tile([1, 1], f32)
                nc.vector.memset(dume[:], 0.0)
                exp_warm = singles.tile([1, 1], f32)
                nc.scalar.activation(exp_warm[:], dume[:], AF.Exp)

        # scores PSUM tile, alive for the whole kernel
        psc = ps_sc.tile([128, NB], f32, tag="psc")
        expm = singles.tile([128, NB], f32)
        # exp/out piece boundaries (in groups); piece p is emitted as soon as
        # the v-dots for its last group are in, so output DMAs overlap compute.
        # Few pieces (exp reads the psc bank the v-dots still write — port
        # contention), and a tiny final piece to shorten the end drain.
        bnd = sorted({NG // 2, max(1, NG - 3), NG})
        exp_after = {b - 1: (bnd[i - 1] if i else 0, b) for i, b in enumerate(bnd)}

        # ---- hot loop, software-pipelined: group g's v-dots are emitted
        # VLAG groups after its relu so the PE never waits on the relu engines.
        pending = []

        def emit_vdot(g, en):
            for k in range(4):
                nc.tensor.matmul(
                    out=psc[:, 4 * g + k : 4 * g + k + 1],
                    lhsT=en[:, 128 * k : 128 * (k + 1)],
                    rhs=vvec_sb,
                    start=True,
                    stop=True,
                )
            if g in exp_after:
                a, b = exp_after[g]
                nc.scalar.activation(expm[:, 4 * a : 4 * b], psc[:, 4 * a : 4 * b], AF.Exp)
                nc.sync.dma_start(out[:, 4 * a : 4 * b], expm[:, 4 * a : 4 * b])

        for g in range(NG):
            pe = ps_e.tile([H, GRP], f32, tag="pe")
            nc.tensor.matmul(
                out=pe[:],
                lhsT=w2t_sb,
                rhs=enc_sb[:, CC + GRP * g : CC + GRP * (g + 1)],
                start=True,
                stop=True,
            )
            en = enpool.tile([H, GRP], bf16, tag="en")
            if g >= NG - 2:
                # drain: split the last relus across both engines for latency
                nc.scalar.activation(
                    en[:, : GRP // 2], pe[:, : GRP // 2], AF.Relu,
                    bias=hpb_sb[:, g : g + 1],
                )
                nc.vector.tensor_scalar(
                    out=en[:, GRP // 2 :],
                    in0=pe[:, GRP // 2 :],
                    scalar1=hpb_sb[:, g : g + 1],
                    scalar2=0.0,
                    op0=mybir.AluOpType.add,
                    op1=mybir.AluOpType.max,
                )
            elif g % 2 == 1:
                nc.scalar.activation(
                    en[:], pe[:], AF.Relu, bias=hpb_sb[:, g : g + 1]
                )
            else:
                nc.vector.tensor_scalar(
                    out=en[:],
                    in0=pe[:],
                    scalar1=hpb_sb[:, g : g + 1],
                    scalar2=0.0,
                    op0=mybir.AluOpType.add,
                    op1=mybir.AluOpType.max,
                )
            pending.append((g, en))
            if len(pending) > VLAG:
                pg, pen = pending.pop(0)
                emit_vdot(pg, pen)
        for pg, pen in pending:
            emit_vdot(pg, pen)


def _make_inmaps(enc, ls, hproj, W_attn, v, NG, core_groups):
    bf = _np_bf16()
    CC = H + 1
    in_maps = []
    for k in range(NCORES):
        e = np.zeros((H, CC + GRP * NG), bf)
        e[:, :H] = W_attn[:, H:].T.astype(bf)  # w2t
        e[:, H] = v.astype(bf)
        hb = np.zeros((128, NG), np.float32)
        for g, item in enumerate(core_groups[k]):
            if item is None:
                continue
            r, j = item
            a, b = GRP * j, min(GRP * (j + 1), int(ls[r]))
            e[:, CC + GRP * g : CC + GRP * g + (b - a)] = enc[a:b, r, :].T.astype(bf)
            hb[:, g] = hproj[r]
        in_maps.append({"enc": e, "hpb": hb})
    return in_maps


def run(inputs, trace=False, **spmd_kwargs):
    hidden = np.asarray(inputs["hidden"], dtype=np.float32)
    enc = np.asarray(inputs["encoder_outputs"], dtype=np.float32)
    ls = np.asarray(inputs["len_seq"]).astype(np.int64)
    W_attn = np.asarray(inputs["W_attn"], dtype=np.float32)
    b_attn = np.asarray(inputs["b_attn"], dtype=np.float32)
    v = np.asarray(inputs["v"], dtype=np.float32)
    t_len = enc.shape[0]

    # host-side projection of the tiny [B,H] operand (f64 for accuracy)
    hproj = (
        hidden.astype(np.float64) @ W_attn[:, :H].astype(np.float64).T
        + b_attn.astype(np.float64)
    ).astype(np.float32)

    NG, core_groups = _plan(ls)
    nc = bacc.Bacc("TRN2", target_bir_lowering=False, debug=False)
    _build(nc, NG)
    nc.compile()
    in_maps = _make_inmaps(enc, ls, hproj, W_attn, v, NG, core_groups)
    res = run_bass_kernel_spmd(
        nc, in_maps, core_ids=list(range(NCORES)), trace=trace, **spmd_kwargs
    )

    # gather: per-core exp values -> per-row f64 normalize -> [B, 1, T]
    flat = [
        np.ascontiguousarray(
            np.asarray(res.results[k]["out"], dtype=np.float32).T
        ).reshape(-1)
        for k in range(NCORES)
    ]
    final = np.zeros((B, 1, t_len), dtype=np.float32)
    rowbuf = {r: [] for r in range(B)}
    for k in range(NCORES):
        for g, item in enumerate(core_groups[k]):
            if item is None:
                continue
            r, _ = item
            rowbuf[r].append(flat[k][GRP * g : GRP * (g + 1)])
    for r in range(B):
        ln = int(ls[r])
        e = np.concatenate(rowbuf[r])[:ln].astype(np.float64)
        final[r, 0, :ln] = (e / e.sum()).astype(np.float32)
    return final, res


def kernel(**inputs):
    final, _ = run(inputs, trace=False)
    return final


# revision 30
# speedup vs baseline: 1.0371x; 1.0371x over previous
"""Trainium2 Bass kernel for ragged masked attention-score softmax.

Problem (B=32, T=8192, H=128):
    energy[b,t] = relu(W1 @ hidden[b] + W2 @ enc[t,b] + b_attn)   (W_attn = [W1 | W2])
    scores[b,t] = v . energy[b,t]
    out[b,0,:]  = ragged-masked softmax over t < len_seq[b], zeros after.

Strategy (8 NeuronCores, flat position-parallel):
  - Every row is padded to a 512-position multiple and the resulting stream of
    512-position "groups" is dealt evenly across the 8 cores (NG groups each,
    rows may straddle cores).  All cores run one shared graph.
  - Host precomputes hproj[r] = W1 @ hidden[r] + b_attn (f64) and ships one
    f32 bias column per group, so the device never touches W1/hidden.
  - Device per group: energy = w2t.T @ encT (PE, bf16) -> bias+relu on
    ScalarE/VectorE (alternating per group) -> per-128-block v-dot via PE
    (energy block stationary, v moving) accumulating scores into one PSUM
    tile [128, 4*NG], position-major.
  - Scores are bounded (|s| <~ 8) so no max subtraction is needed: the device
    applies exp directly (f32) and DMAs the raw exp values out.  The host
    gather divides each row by its f64 sum - exact softmax, and rows split
    across cores combine for free.
  - A ~3.4us chain of dummy matmuls at graph start releases the PE HAM clock
    gate (1.2 -> 2.4 GHz) during the initial DMA window, so the real stream
    runs warm from its first instruction.
"""

from contextlib import ExitStack

import numpy as np

import concourse.bass as bass
import concourse.tile as tile
from concourse import bacc, mybir
from concourse.bass_utils import run_bass_kernel_spmd

B, T, H = 32, 8192, 128
NCORES = 8
GRP = 512  # positions per group (max moving free dim / PSUM bank)
VLAG = 2  # groups of lag between relu and v-dot emission
NWARM = 8  # dummy warm-up matmuls (~8 * 427ns cold = 3.4us)
NOUT = 4  # exp/output DMA pieces
CHW = 1024  # enc DMA chunk width (columns)


def _np_bf16():
    import ml_dtypes

    return np.dtype(ml_dtypes.bfloat16)


def _plan(ls):
    """Lay out the global stream of 512-position groups and deal to cores.

    Returns (NG, core_groups) where core_groups[k] is a list of length NG of
    (row, j) tuples - group j of row `row` covers positions [512j, 512j+512)
    - with None entries for padding groups.
    """
    glist = []
    for r in range(B):
        for j in range((int(ls[r]) + GRP - 1) // GRP):
            glist.append((r, j))
    ng_tot = len(glist)
    NG = (ng_tot + NCORES - 1) // NCORES
    npad = NG * NCORES - ng_tot
    # spread the padding groups over the tails of the last `npad` cores
    core_groups = []
    pos = 0
    for k in range(NCORES):
        take = NG - (1 if k >= NCORES - npad else 0)
        gs = glist[pos : pos + take] + [None] * (NG - take)
        pos += take
        core_groups.append(gs)
    return NG, core_groups


def _build(nc, NG):
    """Emit the shared Tile graph for one core (NG groups of 512 positions)."""
    bf16 = mybir.dt.bfloat16
    f32 = mybir.dt.float32
    AF = mybir.ActivationFunctionType
    NB = 4 * NG  # number of 128-position blocks
    CC = H + 1  # const columns (w2t | v) packed in front of enc
    NC = CC + GRP * NG  # total enc columns incl consts

    enc = nc.dram_tensor("enc", [H, NC], bf16, kind="ExternalInput").ap()
    hpb = nc.dram_tensor("hpb", [128, NG], f32, kind="ExternalInput").ap()
    out = nc.dram_tensor("out", [128, NB], f32, kind="ExternalOutput").ap()

    with ExitStack() as ctx:
        tc = ctx.enter_context(tile.TileContext(nc))
        singles = ctx.enter_context(tc.tile_pool(name="singles", bufs=1))
        enpool = ctx.enter_context(tc.tile_pool(name="energy", bufs=8))
        ps_e = ctx.enter_context(tc.tile_pool(name="ps_e", bufs=5, space="PSUM"))
        ps_sc = ctx.enter_context(tc.tile_pool(name="ps_sc", bufs=1, space="PSUM"))
        ps_w = ctx.enter_context(tc.tile_pool(name="ps_w", bufs=1, space="PSUM"))

        # ---- PE warm-up: ~3.4us of dense dummy matmuls during the DMA wait
        # window flips the HAM clock gate to 2.4 GHz before the real stream.
        dum = singles.tile([H, H], bf16)
        nc.vector.memset(dum[:], 0.0)
        dumr = singles.tile([H, GRP], bf16)
        nc.vector.memset(dumr[:], 0.0)
        pw = ps_w.tile([H, GRP], f32, tag="warm")
        for _ in range(NWARM):
            nc.tensor.matmul(out=pw[:], lhsT=dum[:], rhs=dumr[:], start=True, stop=True)

        # ---- DMAs: uniform chunks at a steady cadence; the consts ride in
        # front of chunk 0 so one semaphore covers both.  Compute is paced to
        # trail the chunk-semaphore wave by a constant margin (see NWARM).
        enc_sb = singles.tile([H, NC], bf16)
        w2t_sb = enc_sb[:, :H]
        vvec_sb = enc_sb[:, H : H + 1]
        # small chunks early (dense, early semaphore wave for the pipeline
        # head), wider chunks later where the wave is latency-pipelined, and
        # short tail chunks for a quick final semaphore
        chunks = []
        c0 = 0
        while c0 < NC:
            rem = NC - c0
            if c0 == 0:
                cw = CC + GRP
            elif len(chunks) < 8:
                cw = min(GRP, rem)
            elif rem <= 2 * GRP:
                cw = min(GRP, rem)
            else:
                cw = min(CHW, rem - 2 * GRP)
            chunks.append((c0, cw))
            c0 += cw
        # chunk 0 rides GpSimd's SWDGE: GpSimd clears the framework preamble
        # ~1.3us before Scalar/Sync, so the stream starts that much earlier.
        # The next few issues alternate Scalar/Sync HWDGE rings: each ring's
        # ~0.6us per-issue cost would otherwise cap the early semaphore wave.
        hpb_sb = singles.tile([128, NG], f32)
        for i, (c0, cw) in enumerate(chunks):
            if i == 0:
                eng = nc.gpsimd
            elif i < 8:
                eng = nc.scalar if i % 2 == 1 else nc.sync
            else:
                eng = nc.sync
            eng.dma_start(enc_sb[:, c0 : c0 + cw], enc[:, c0 : c0 + cw])
            if i == 0:
                nc.scalar.dma_start(hpb_sb[:], hpb[:])
                # preload the exp ACT table (1.3us) after ScalarE's first DMA
                # issues but before its first relu, overlapping the enc stream
                dume = singles.tile([1, 1], f32)
                nc.vector.memset(dume[:], 0.0)
                exp_warm = singles.tile([1, 1], f32)
                nc.scalar.activation(exp_warm[:], dume[:], AF.Exp)

        # scores PSUM tile, alive for the whole kernel
        psc = ps_sc.tile([128, NB], f32, tag="psc")
        expm = singles.tile([128, NB], f32)
        # exp/out piece boundaries (in groups); piece p is emitted as soon as
        # the v-dots for its last group are in, so output DMAs overlap compute.
        # Few pieces (exp reads the psc bank the v-dots still write — port
        # contention), and a tiny final piece to shorten the end drain.
        bnd = sorted({NG // 2, max(1, NG - 3), NG})
        exp_after = {b - 1: (bnd[i - 1] if i else 0, b) for i, b in enumerate(bnd)}

        # ---- hot loop, software-pipelined: group g's v-dots are emitted
        # VLAG groups after its relu so the PE never waits on the relu engines.
        pending = []

        def emit_vdot(g, en):
            for k in range(4):
                nc.tensor.matmul(
                    out=psc[:, 4 * g + k : 4 * g + k + 1],
                    lhsT=en[:, 128 * k : 128 * (k + 1)],
                    rhs=vvec_sb,
                    start=True,
                    stop=True,
                )
            if g in exp_after:
                a, b = exp_after[g]
                nc.scalar.activation(expm[:, 4 * a : 4 * b], psc[:, 4 * a : 4 * b], AF.Exp)
                # the final piece's DMA issues from ScalarE right after its
                # exp - no cross-engine semaphore hop on the kernel's tail
                eng = nc.scalar if b == NG else nc.sync
                eng.dma_start(out[:, 4 * a : 4 * b], expm[:, 4 * a : 4 * b])

        for g in range(NG):
            pe = ps_e.tile([H, GRP], f32, tag="pe")
            nc.tensor.matmul(
                out=pe[:],
                lhsT=w2t_sb,
                rhs=enc_sb[:, CC + GRP * g : CC + GRP * (g + 1)],
                start=True,
                stop=True,
            )
            en = enpool.tile([H, GRP], bf16, tag="en")
            if g >= NG - 2:
                # drain: split the last relus across both engines for latency
                nc.scalar.activation(
                    en[:, : GRP // 2], pe[:, : GRP // 2], AF.Relu,
                    bias=hpb_sb[:, g : g + 1],
                )
                nc.vector.tensor_scalar(
                    out=en[:, GRP // 2 :],
                    in0=pe[:, GRP // 2 :],
                    scalar1=hpb_sb[:, g : g + 1],
                    scalar2=0.0,
                    op0=mybir.AluOpType.add,
                    op1=mybir.AluOpType.max,
                )
            elif g % 2 == 1:
                nc.scalar.activation(
                    en[:], pe[:], AF.Relu, bias=hpb_sb[:, g : g + 1]
                )
            else:
                nc.vector.tensor_scalar(
                    out=en[:],
                    in0=pe[:],
                    scalar1=hpb_sb[:, g : g + 1],
                    scalar2=0.0,
                    op0=mybir.AluOpType.add,
                    op1=mybir.AluOpType.max,
                )
            pending.append((g, en))
            if len(pending) > VLAG:
                pg, pen = pending.pop(0)
                emit_vdot(pg, pen)
        for pg, pen in pending:
            emit_vdot(pg, pen)


def _make_inmaps(enc, ls, hproj, W_attn, v, NG, core_groups):
    bf = _np_bf16()
    CC = H + 1
    in_maps = []
    for k in range(NCORES):
        e = np.zeros((H, CC + GRP * NG), bf)
        e[:, :H] = W_attn[:, H:].T.astype(bf)  # w2t
        e[:, H] = v.astype(bf)
        hb = np.zeros((128, NG), np.float32)
        for g, item in enumerate(core_groups[k]):
            if item is None:
                continue
            r, j = item
            a, b = GRP * j, min(GRP * (j + 1), int(ls[r]))
            e[:, CC + GRP * g : CC + GRP * g + (b - a)] = enc[a:b, r, :].T.astype(bf)
            hb[:, g] = hproj[r]
        in_maps.append({"enc": e, "hpb": hb})
    return in_maps


def run(inputs, trace=False, **spmd_kwargs):
    hidden = np.asarray(inputs["hidden"], dtype=np.float32)
    enc = np.asarray(inputs["encoder_outputs"], dtype=np.float32)
    ls = np.asarray(inputs["len_seq"]).astype(np.int64)
    W_attn = np.asarray(inputs["W_attn"], dtype=np.float32)
    b_attn = np.asarray(inputs["b_attn"], dtype=np.float32)
    v = np.asarray(inputs["v"], dtype=np.float32)
    t_len = enc.shape[0]

    # host-side projection of the tiny [B,H] operand (f64 for accuracy)
    hproj = (
        hidden.astype(np.float64) @ W_attn[:, :H].astype(np.float64).T
        + b_attn.astype(np.float64)
    ).astype(np.float32)

    NG, core_groups = _plan(ls)
    nc = bacc.Bacc("TRN2", target_bir_lowering=False, debug=False)
    _build(nc, NG)
    nc.compile()
    in_maps = _make_inmaps(enc, ls, hproj, W_attn, v, NG, core_groups)
    res = run_bass_kernel_spmd(
        nc, in_maps, core_ids=list(range(NCORES)), trace=trace, **spmd_kwargs
    )

    # gather: per-core exp values -> per-row f64 normalize -> [B, 1, T]
    flat = [
        np.ascontiguousarray(
            np.asarray(res.results[k]["out"], dtype=np.float32).T
        ).reshape(-1)
        for k in range(NCORES)
    ]
    final = np.zeros((B, 1, t_len), dtype=np.float32)
    rowbuf = {r: [] for r in range(B)}
    for k in range(NCORES):
        for g, item in enumerate(core_groups[k]):
            if item is None:
                continue
            r, _ = item
            rowbuf[r].append(flat[k][GRP * g : GRP * (g + 1)])
    for r in range(B):
        ln = int(ls[r])
        e = np.concatenate(rowbuf[r])[:ln].astype(np.float64)
        final[r, 0, :ln] = (e / e.sum()).astype(np.float32)
    return final, res


def kernel(**inputs):
    final, _ = run(inputs, trace=False)
    return final
